# revision 2
# baseline (speedup 1.0000x reference)
"""Trainium2 Bass kernel for nn_ComparisonLayer (v2: 2-o pack drains).

Computes, for x:(L,B,D) with L=512,B=2,D=256,C=128,O=64:
    xb  = layernorm(transpose(x,(1,0,2)))          # (B,L,D)
    a   = xb@w1+b1 ; b = xb@w2+b2                  # (B,L,C)
    out[b,l,m,o] = sum_c a[b,l,c]*b[b,m,c]*w3[c,o] + b3[o]
                   + (a@w4)[b,l,o] - (b@w4)[b,m,o] # (B,L,L,O)

Sharding: 8 cores, core k handles batch k//4 and l-block q=k%4.

Structure per core (all row terms folded into one K=2 rank matmul):
    out[l, o, m] = (aT * w3[:,o]).T @ bT           # main MM, K=128 N=512
                 + ones(l) x negb4T[o, m]          # K=2 rank MM rows:
                 + a4T[o, l] x ones(m)             #   [ones; a4] x [negb4T; ones]
Drains are pure PSUM->SBUF copies of 2-o (2-bank) packs, split ACT/DVE.
b4T is computed directly from xn via host-folded w24 = w2g @ w4, so the
rank-term chain does not wait for bT.  Input x is bf16; row tiles are
rotated so the core's own l-block (tile q) lands and processes first.
Device output layout (l, (o, m_dev)) bf16; host un-rotates m and
transposes to (l, m, o) fp32.
"""

import numpy as np
import ml_dtypes

import concourse.bacc as bacc
import concourse.bass as bass
import concourse.mybir as mybir
import concourse.tile as tile
from concourse.bass_utils import run_bass_kernel_spmd

L, B, D, C, O = 512, 2, 256, 128, 64
NCORES = 8
LBLK = 128                   # l rows per core
NT = 4                       # row tiles of 128
LN_EPS = 1e-5                # folded away (negligible vs var ~ 1)

F32 = mybir.dt.float32
BF16 = mybir.dt.bfloat16

# wbfa (bf16) columns: identity + w1 halves
WA_ID = 0            # [0:128)
WA_W1 = 128          # [128:384)  w1g halves (h p) c -> p (h c)
WA_N = 384
# wbfb (bf16) columns: w2 halves + w24 halves + w4
WB_W2 = 0            # [0:256)
WB_W24 = 256         # [256:384)  w24 halves (h p) o -> p (h o)
WB_W4 = 384          # [384:448)  w4 (C, O)
WB_N = 448
# wf32 columns
WF_W3 = 0            # [0:64)   w3 (C, O)
WF_B1 = 64           # b1e
WF_B2 = 65           # b2e
WF_NB = 66           # nbias = b3 - b2e@w4, partitions 0..63
WF_N = 67

# output blocks (o ranges per DMA)
BOUNDS = [0, 4, 12, 20, 28, 36, 44, 52, 58, 62, 64]
OBLK_MAX = max(b - a for a, b in zip(BOUNDS, BOUNDS[1:]))


def _build():
    nc = bacc.Bacc("TRN2", target_bir_lowering=False, debug=False)

    xall_d = nc.dram_tensor("xall", (128, NT, D), BF16, kind="ExternalInput")
    wbfa_d = nc.dram_tensor("wbfa", (128, WA_N), BF16, kind="ExternalInput")
    wbfb_d = nc.dram_tensor("wbfb", (128, WB_N), BF16, kind="ExternalInput")
    wf32_d = nc.dram_tensor("wf32", (128, WF_N), F32, kind="ExternalInput")
    out_d = nc.dram_tensor("out", (LBLK, O * L), BF16, kind="ExternalOutput")

    AX = mybir.AxisListType.X
    ALU = mybir.AluOpType
    ACT = mybir.ActivationFunctionType

    with tile.TileContext(nc) as tc:
        with (
            tc.tile_pool(name="const", bufs=1) as cp,
            tc.tile_pool(name="work", bufs=2) as wp,
            tc.tile_pool(name="aw", bufs=12) as awp,
            tc.tile_pool(name="ob", bufs=3) as obp,
            tc.tile_pool(name="ps", bufs=3, space="PSUM") as pm,
            tc.tile_pool(name="tp", bufs=2, space="PSUM") as tpm,
        ):
            # ---------- input DMAs (per-tile split, earliest-needed first) --
            xall = cp.tile([128, NT, D], BF16)
            wbfa = cp.tile([128, WA_N], BF16)
            wbfb = cp.tile([128, WB_N], BF16)
            wf32 = cp.tile([128, WF_N], F32)
            nc.sync.dma_start(xall[:, 0, :], xall_d[:, 0, :])
            nc.sync.dma_start(wbfa[:], wbfa_d[:])
            nc.sync.dma_start(xall[:, 1, :], xall_d[:, 1, :])
            nc.sync.dma_start(wbfb[:], wbfb_d[:])
            nc.sync.dma_start(wf32[:], wf32_d[:])
            nc.sync.dma_start(xall[:, 2, :], xall_d[:, 2, :])
            nc.sync.dma_start(xall[:, 3, :], xall_d[:, 3, :])

            id128 = wbfa[:, WA_ID:WA_ID + 128]
            w4s = wbfb[:, WB_W4:WB_W4 + O]

            # ---------- constants / replication targets ----------
            onesb = cp.tile([128, 512], BF16)
            nc.vector.memset(onesb[:], 1.0)
            # rowRep: partition 32r   <- negb4T rows of strip r (flattened)
            #         partition 32r+1 <- ones (from memset)
            rowRep = cp.tile([128, 16 * L], BF16)
            nc.gpsimd.memset(rowRep[:], 1.0)
            # rk: partition 32r <- ones, partition 32r+1 <- a4T strip r flat
            rk = cp.tile([128, 16 * 128], BF16)
            nc.gpsimd.memset(rk[:], 1.0)
            # dummy 1-col Sqrt pulls the ACT table load off the LN chain
            tblw = wp.tile([128, 1], F32, tag="tblw")
            nc.vector.memset(tblw[:], 1.0)
            nc.scalar.activation(tblw[:], tblw[:], ACT.Sqrt)

            # long-lived PSUM accumulators (allocate first: pin ring slots)
            bps_t = pm.tile([128, 2 * L], F32, tag="ps", name="bps")
            bps = bps_t[:, 0:L]
            b4ps_t = pm.tile([128, 2 * L], F32, tag="ps", name="b4ps")
            b4ps = b4ps_t[0:O, 0:L]

            # ---------- HAM warm-up burst ----------
            def warm(n, nm):
                for wi in range(n):
                    wps = pm.tile([128, 2 * L], F32, tag="ps", name=f"{nm}{wi}")
                    nc.tensor.matmul(wps[:, 0:L], onesb[:, 0:128],
                                     onesb[:, 0:L], start=True, stop=True)

            warm(6, "wburst")

            # ---------- per-tile LN + transpose + bT/b4 matmuls ----------
            ssum = wp.tile([128, NT], F32, tag="ssum")
            vs = wp.tile([128, NT], F32, tag="vs")
            nmu2 = wp.tile([128, NT], F32, tag="nmu2")
            std = wp.tile([128, NT], F32, tag="std")
            rstd = wp.tile([128, NT], F32, tag="rstd")
            nmrs = wp.tile([128, NT], F32, tag="nmrs")
            xn = cp.tile([128, NT, D], BF16)
            xnT = cp.tile([128, 2, NT, 128], BF16)
            aT_c = cp.tile([C, 128], BF16)
            a4bf = cp.tile([O, 128], BF16)
            bT_c = cp.tile([C, L], BF16)
            nb = cp.tile([O, L], BF16)

            for s in range(NT):
                xs = xall[:, s, :]
                nc.vector.tensor_reduce(
                    ssum[:, s:s + 1], xs, axis=AX, op=ALU.add)
                sq = wp.tile([128, D], F32, tag="sq")
                nc.scalar.activation(
                    sq[:], xs, ACT.Square, accum_out=vs[:, s:s + 1])
                nc.vector.scalar_tensor_tensor(
                    nmu2[:, s:s + 1], ssum[:, s:s + 1], -1.0 / (D * D),
                    ssum[:, s:s + 1], op0=ALU.mult, op1=ALU.mult)
                nc.scalar.activation(
                    std[:, s:s + 1], vs[:, s:s + 1], ACT.Sqrt,
                    bias=nmu2[:, s:s + 1], scale=1.0 / D)
                nc.vector.reciprocal(rstd[:, s:s + 1], std[:, s:s + 1])
                nc.vector.scalar_tensor_tensor(
                    nmrs[:, s:s + 1], ssum[:, s:s + 1], -1.0 / D,
                    rstd[:, s:s + 1], op0=ALU.mult, op1=ALU.mult)
                nc.vector.tensor_scalar(
                    xn[:, s, :], xs, rstd[:, s:s + 1], nmrs[:, s:s + 1],
                    op0=ALU.mult, op1=ALU.add)
                for h in range(2):
                    tp = tpm.tile([128, 512], BF16, tag="tp", name=f"tp{s}{h}")
                    nc.tensor.transpose(
                        tp[:, 0:128], xn[:, s, h * 128:(h + 1) * 128], id128)
                    if h == 0:
                        nc.scalar.copy(xnT[:, h, s, :], tp[:, 0:128])
                    else:
                        nc.vector.tensor_copy(xnT[:, h, s, :], tp[:, 0:128])
                # bT rows: accumulate w2g.T @ xnT into bps columns of tile s
                for h in range(2):
                    nc.tensor.matmul(
                        bps[:, s * 128:(s + 1) * 128],
                        wbfb[:, WB_W2 + h * 128:WB_W2 + (h + 1) * 128],
                        xnT[:, h, s, :],
                        start=(s == 0 and h == 0), stop=(s == 3 and h == 1),
                    )
                # b4T rows: accumulate w24.T @ xnT (direct, skips bT)
                for h in range(2):
                    nc.tensor.matmul(
                        b4ps[:, s * 128:(s + 1) * 128],
                        wbfb[:, WB_W24 + h * O:WB_W24 + (h + 1) * O],
                        xnT[:, h, s, :],
                        start=(s == 0 and h == 0), stop=(s == 3 and h == 1),
                    )
                if s == 0:
                    # aT / a4T chain (tile 0 == this core's own l-block)
                    aps_t = pm.tile([128, 2 * L], F32, tag="ps", name="aps")
                    for h in range(2):
                        nc.tensor.matmul(
                            aps_t[:, 0:128],
                            wbfa[:, WA_W1 + h * 128:WA_W1 + (h + 1) * 128],
                            xnT[:, h, 0, :], start=(h == 0), stop=(h == 1),
                        )
                    nc.vector.tensor_scalar_add(
                        aT_c[:], aps_t[:, 0:128], wf32[:, WF_B1:WF_B1 + 1])
                    a4ps_t = pm.tile([128, 2 * L], F32, tag="ps", name="a4ps")
                    nc.tensor.matmul(a4ps_t[0:O, 0:128], w4s, aT_c[:],
                                     start=True, stop=True)
                    nc.scalar.copy(a4bf[:], a4ps_t[0:O, 0:128])
                    # a4T strip replicas -> rk partitions 32r+1
                    for r in range(4):
                        eng = nc.scalar if r % 2 == 0 else nc.sync
                        eng.dma_start(
                            rk[32 * r + 1:32 * r + 2, :], a4bf[r:O:4, :])
                    warm(1, "wa")

            # bT_c (bias add) gates the main matmuls
            nc.vector.tensor_scalar_add(
                bT_c[:], bps[:], wf32[:, WF_B2:WF_B2 + 1])
            # negb4T = -b4 + (b3 - b2e@w4); flatten strips into rowRep
            nc.scalar.activation(
                nb[:], b4ps[:], ACT.Identity,
                bias=wf32[0:O, WF_NB:WF_NB + 1], scale=-1.0)
            for r in range(4):
                eng = nc.scalar if r % 2 == 0 else nc.sync
                eng.dma_start(rowRep[32 * r:32 * r + 1, :], nb[r:O:4, :])
            warm(2, "wrr")

            # ---------- main loop: 16 groups of 4 o's, 2-o PSUM packs ------
            blk_of = {}
            for bs, be in zip(BOUNDS, BOUNDS[1:]):
                for o in range(bs, be):
                    blk_of[o] = (bs, be)

            aw_tiles = {}

            def build_aw(o):
                t = awp.tile([C, 128], BF16, tag="aw", name=f"aw{o}")
                nc.vector.tensor_scalar_mul(
                    t[:], aT_c[:], wf32[:, WF_W3 + o:WF_W3 + o + 1])
                aw_tiles[o] = t

            for o in range(8):
                build_aw(o)

            ob = None
            for g in range(O // 4):
                if g + 2 < O // 4:
                    for j in range(4):
                        build_aw(4 * (g + 2) + j)
                packA = pm.tile([128, 2 * L], F32, tag="ps", name=f"pkA{g}")
                packB = pm.tile([128, 2 * L], F32, tag="ps", name=f"pkB{g}")
                packs = (packA, packB)
                # main matmuls first (start=True)
                for j in range(4):
                    o = 4 * g + j
                    dst = packs[j // 2][:, (j % 2) * L:(j % 2 + 1) * L]
                    nc.tensor.matmul(dst, aw_tiles.pop(o)[:], bT_c[:],
                                     start=True, stop=False)
                # rank matmuls: [ones; a4T[o]] x [negb4T[o]; ones]
                for j in range(4):
                    o = 4 * g + j
                    dst = packs[j // 2][:, (j % 2) * L:(j % 2 + 1) * L]
                    nc.tensor.matmul(
                        dst,
                        rk[32 * j:32 * j + 2, g * 128:(g + 1) * 128],
                        rowRep[32 * j:32 * j + 2, g * L:(g + 1) * L],
                        start=False, stop=True, tile_position=(32 * j, 0),
                    )
                # drains: 2-o pure copies, ACT:DVE = 5:3
                for half in range(2):
                    p_idx = 2 * g + half
                    o0 = 2 * p_idx
                    bs, be = blk_of[o0]
                    if o0 == bs:
                        ob = obp.tile([128, OBLK_MAX * L], BF16, tag="ob",
                                      name=f"ob{p_idx}")
                    sl = ob[:, (o0 - bs) * L:(o0 - bs + 2) * L]
                    if p_idx % 8 in (0, 2, 4, 6, 7):
                        nc.scalar.copy(sl, packs[half][:])
                    else:
                        nc.vector.tensor_copy(sl, packs[half][:])
                    if o0 + 2 == be:
                        nc.sync.dma_start(
                            out_d[:, bs * L:be * L], ob[:, 0:(be - bs) * L])

    nc.compile()
    return nc


_CACHE = {}


def _get_nc():
    if "nc" not in _CACHE:
        _CACHE["nc"] = _build()
    return _CACHE["nc"]


def _make_in_maps(x, ln_gamma, ln_beta, w1, b1, w2, b2, w3, b3, w4):
    x = np.ascontiguousarray(x, dtype=np.float32)
    g = np.asarray(ln_gamma, np.float32)
    be = np.asarray(ln_beta, np.float32)
    w1 = np.asarray(w1, np.float32)
    w2 = np.asarray(w2, np.float32)
    # fold the LN affine into the first-layer weights:
    # (xn*g + be) @ w = xn @ (g[:,None]*w) + be @ w
    w1g = g[:, None] * w1
    w2g = g[:, None] * w2
    b1e = np.asarray(b1, np.float32) + be @ w1
    b2e = np.asarray(b2, np.float32) + be @ w2
    w3c = np.asarray(w3, np.float32)
    w4f = np.asarray(w4, np.float32)
    b3f = np.asarray(b3, np.float32)
    w24 = w2g @ w4f                      # (D, O)
    nbias = b3f - b2e @ w4f              # (O,)

    bf = ml_dtypes.bfloat16
    wbfa = np.zeros((128, WA_N), dtype=bf)
    wbfa[:, WA_ID:WA_ID + 128] = np.eye(128, dtype=np.float32).astype(bf)
    wbfa[:, WA_W1:WA_W1 + 256] = \
        w1g.reshape(2, 128, C).transpose(1, 0, 2).reshape(128, 256).astype(bf)

    wbfb = np.zeros((128, WB_N), dtype=bf)
    wbfb[:, WB_W2:WB_W2 + 256] = \
        w2g.reshape(2, 128, C).transpose(1, 0, 2).reshape(128, 256).astype(bf)
    wbfb[:, WB_W24:WB_W24 + 128] = \
        w24.reshape(2, 128, O).transpose(1, 0, 2).reshape(128, 128).astype(bf)
    wbfb[:, WB_W4:WB_W4 + O] = w4f.astype(bf)

    wf32 = np.zeros((128, WF_N), dtype=np.float32)
    wf32[:, WF_W3:WF_W3 + O] = w3c
    wf32[:, WF_B1] = b1e
    wf32[:, WF_B2] = b2e
    wf32[0:O, WF_NB] = nbias

    in_maps = []
    for k in range(NCORES):
        bi, q = k // (NCORES // B), k % (NCORES // B)
        xb = x[:, bi, :]                                    # (L, D)
        xtiles = xb.reshape(NT, 128, D)                     # (NT, 128, D)
        # rotate: slot s holds input tile (q+s)%NT; slot 0 = own l-block
        order = [(q + s) % NT for s in range(NT)]
        xall = np.ascontiguousarray(
            xtiles[order].transpose(1, 0, 2).astype(bf))    # (128, NT, D)
        in_maps.append(
            {"xall": xall, "wbfa": wbfa, "wbfb": wbfb, "wf32": wf32})
    return in_maps


def kernel_run(inputs, trace=False):
    nc = _get_nc()
    in_maps = _make_in_maps(**inputs)
    res = run_bass_kernel_spmd(
        nc, in_maps, core_ids=list(range(NCORES)), trace=trace,
    )
    out = np.empty((B, L, L, O), dtype=np.float32)
    for k in range(NCORES):
        bi, q = k // (NCORES // B), k % (NCORES // B)
        blk = np.asarray(res.results[k]["out"]).astype(np.float32)
        tmp = blk.reshape(LBLK, O, NT, 128)      # (l, o, slot, j)
        rows = slice(q * LBLK, (q + 1) * LBLK)
        for s in range(NT):
            t = (q + s) % NT
            out[bi, rows, t * 128:(t + 1) * 128, :] = \
                tmp[:, :, s, :].transpose(0, 2, 1)
    return out, res


def kernel(**inputs) -> np.ndarray:
    out, _ = kernel_run(inputs, trace=False)
    return out
